# revision 5
# baseline (speedup 1.0000x reference)
"""Causal single-head attention on 8 Trainium2 NeuronCores.

Problem: x:[4,2048,1024] f32, W_q/W_k/W_v:[1024,1024] f32.
  q,k,v = x@W; scores = q@k^T/sqrt(d) causal-masked; out = softmax(scores)@v.

Sharding: 8 cores = 4 batches x 2 query-shards (SPMD, identical program,
per-core data). Causal load balance: the 16 query blocks (128 rows each) of a
batch are split between its 2 cores as evens/odds of a pairing chosen so both
cores share one uniform per-pair key-block-count profile [4,8,12,16]
(optimal: 40 key-block iterations/core vs 64 dense).

Layout trick: everything is computed via out = lhsT.T @ rhs with x fed
PRE-TRANSPOSED from the host (xT = x[b].T), so the kernel needs no on-chip
transposes at all:
  K^T[d,s] = Wk_blk.T @ xT     (lhsT = Wk block, natural layout)
  Q^T[d,q] = Wq_blk.T @ xTq
  V[s,d]   = xT_blk.T @ Wv
  S^T[k,q] = K^T_blk.T @ Q^T   (scores transposed: softmax key-dim = partition)
  P^T      = exp(S^T/32) * mask      (multiplicative post-exp causal mask, host data)
  denom[q] = P^T_blk.T @ ones  ([q,1] per-partition layout for free)
  out[q,d] = P^T_blk.T @ V     (PSUM-accumulated over key blocks)
  out     *= 1/denom           (per-partition broadcast)

All matmul inputs bf16 (1 cycle/row, FWL), f32 PSUM accumulation.
"""

import numpy as np
import ml_dtypes

import concourse.mybir as mybir
import concourse.tile as tile
from concourse import bacc
from concourse.bass_utils import run_bass_kernel_spmd
from contextlib import ExitStack

P = 128
S = 2048
D = 1024
NIB = D // P  # 8 contraction blocks
NSB = S // P  # 16 key blocks
NQB = 8  # local query blocks per core
CNT = [4, 8, 12, 16]  # key blocks per query-block pair (uniform across cores)
G_EVEN = [0, 2, 4, 6, 9, 11, 13, 15]
G_ODD = [1, 3, 5, 7, 8, 10, 12, 14]
BF = mybir.dt.bfloat16
F32 = mybir.dt.float32
SCALE = 1.0 / 32.0  # 1/sqrt(1024)
bf16 = ml_dtypes.bfloat16

_prog_cache = {}


def _build_program():
    if "nc" in _prog_cache:
        return _prog_cache["nc"]
    nc = bacc.Bacc("TRN2", target_bir_lowering=False, debug=False, num_devices=8)
    xT = nc.dram_tensor("xT", [D, S], BF, kind="ExternalInput").ap()
    xTq = nc.dram_tensor("xTq", [D, NQB * P], BF, kind="ExternalInput").ap()
    Wq = nc.dram_tensor("Wq", [D, D], BF, kind="ExternalInput").ap()
    Wk = nc.dram_tensor("Wk", [D, D], BF, kind="ExternalInput").ap()
    Wv = nc.dram_tensor("Wv", [D, D], BF, kind="ExternalInput").ap()
    masks = nc.dram_tensor("masks", [16, P, 2 * P], BF, kind="ExternalInput").ap()
    O = nc.dram_tensor("O", [NQB * P, D], F32, kind="ExternalOutput").ap()

    with tile.TileContext(nc) as tc, ExitStack() as ctx:
        # Persistent SBUF residents
        res = ctx.enter_context(tc.tile_pool(name="res", bufs=1))
        kT = [res.tile([P, S], BF, tag=f"kT{d}", name=f"kT{d}") for d in range(NIB)]
        qT = [res.tile([P, NQB * P], BF, tag=f"qT{d}", name=f"qT{d}") for d in range(NIB)]
        v = [res.tile([P, D], BF, tag=f"v{s}", name=f"v{s}") for s in range(NSB)]
        ones = res.tile([P, 1], BF, tag="ones", name="ones")
        nc.vector.memset(ones[:], 1.0)

        # ---------------- Phase A: projections ----------------
        with ExitStack() as actx:
            xp = actx.enter_context(tc.tile_pool(name="xp", bufs=1))
            wp = actx.enter_context(tc.tile_pool(name="wp", bufs=1))
            aps = actx.enter_context(tc.tile_pool(name="aps", bufs=2, space="PSUM"))

            xt = [xp.tile([P, S], BF, tag=f"x{i}", name=f"x{i}") for i in range(NIB)]
            xtq = [xp.tile([P, NQB * P], BF, tag=f"xq{i}", name=f"xq{i}") for i in range(NIB)]
            wk = [wp.tile([P, D], BF, tag=f"wk{i}", name=f"wk{i}") for i in range(NIB)]
            wq = [wp.tile([P, D], BF, tag=f"wq{i}", name=f"wq{i}") for i in range(NIB)]
            wv = [wp.tile([P, D], BF, tag=f"wv{i}", name=f"wv{i}") for i in range(NIB)]
            for i in range(NIB):
                nc.sync.dma_start(xt[i][:], xT[i * P : (i + 1) * P, :])
                nc.sync.dma_start(wk[i][:], Wk[i * P : (i + 1) * P, :])
                nc.sync.dma_start(xtq[i][:], xTq[i * P : (i + 1) * P, :])
                nc.sync.dma_start(wq[i][:], Wq[i * P : (i + 1) * P, :])
                nc.sync.dma_start(wv[i][:], Wv[i * P : (i + 1) * P, :])

            def proj(dst, dst_col, lhsT, rhs, n512):
                """dst[:, dst_col*512+...] = sum_i lhsT[i].T @ rhs[i], n512 cols"""
                ps = aps.tile([P, 512], F32, tag="aps", name="aps")
                for i in range(NIB):
                    nc.tensor.matmul(
                        ps[:], lhsT[i], rhs[i], start=(i == 0), stop=(i == NIB - 1)
                    )
                nc.vector.tensor_copy(
                    dst[:, dst_col * 512 : dst_col * 512 + 512], ps[:]
                )

            # K^T then Q^T (phase B pair 0 needs them first), then V
            for d in range(NIB):
                for n in range(S // 512):
                    proj(
                        kT[d], n,
                        [wk[i][:, d * P : (d + 1) * P] for i in range(NIB)],
                        [xt[i][:, n * 512 : (n + 1) * 512] for i in range(NIB)],
                        1,
                    )
            for d in range(NIB):
                for n in range(NQB * P // 512):
                    proj(
                        qT[d], n,
                        [wq[i][:, d * P : (d + 1) * P] for i in range(NIB)],
                        [xtq[i][:, n * 512 : (n + 1) * 512] for i in range(NIB)],
                        1,
                    )
            for s in range(NSB):
                for n in range(D // 512):
                    proj(
                        v[s], n,
                        [xt[i][:, s * P : (s + 1) * P] for i in range(NIB)],
                        [wv[i][:, n * 512 : (n + 1) * 512] for i in range(NIB)],
                        1,
                    )

        # ---------------- Phase B: attention ----------------
        mp = ctx.enter_context(tc.tile_pool(name="mp", bufs=1))
        m_tiles = [mp.tile([P, 2 * P], BF, tag=f"m{i}", name=f"m{i}") for i in range(16)]
        for i in range(16):
            nc.sync.dma_start(m_tiles[i][:], masks[i, :, :])

        spool = ctx.enter_context(tc.tile_pool(name="spool", bufs=2, space="PSUM"))
        avpool = ctx.enter_context(tc.tile_pool(name="avpool", bufs=1, space="PSUM"))
        dpool = ctx.enter_context(tc.tile_pool(name="dpool", bufs=1, space="PSUM"))
        pp = ctx.enter_context(tc.tile_pool(name="pp", bufs=3))
        op = ctx.enter_context(tc.tile_pool(name="op", bufs=2))
        rp = ctx.enter_context(tc.tile_pool(name="rp", bufs=2))

        for p in range(4):
            av = [
                [avpool.tile([P, 512], F32, tag=f"av{e}{n}", name=f"av{e}{n}") for n in range(2)]
                for e in range(2)
            ]
            # one PSUM tile (bank) per query-half: matmul start=True clears
            # has_written for the WHOLE bank, so two interleaved accumulation
            # groups cannot share a bank
            den = [
                dpool.tile([P, 1], F32, tag=f"den{e}", name=f"den{e}")
                for e in range(2)
            ]
            for kb in range(CNT[p]):
                ps_s = spool.tile([P, 2 * P], F32, tag="ps_s", name="ps_s")
                for d in range(NIB):
                    nc.tensor.matmul(
                        ps_s[:],
                        kT[d][:, kb * P : (kb + 1) * P],
                        qT[d][:, p * 2 * P : (p + 1) * 2 * P],
                        start=(d == 0),
                        stop=(d == NIB - 1),
                    )
                pT = pp.tile([P, 2 * P], BF, tag="pT", name="pT")
                nc.scalar.activation(
                    pT[:], ps_s[:], mybir.ActivationFunctionType.Exp, scale=SCALE
                )
                if kb >= CNT[p] - 4:
                    mi = p * 4 + kb - (CNT[p] - 4)
                    pTm = pp.tile([P, 2 * P], BF, tag="pTm", name="pTm")
                    nc.vector.tensor_mul(pTm[:], pT[:], m_tiles[mi][:])
                    pT = pTm
                first, last = (kb == 0), (kb == CNT[p] - 1)
                for e in range(2):
                    lhs = pT[:, e * P : (e + 1) * P]
                    for n in range(2):
                        nc.tensor.matmul(
                            av[e][n][:], lhs, v[kb][:, n * 512 : (n + 1) * 512],
                            start=first, stop=last,
                        )
                    nc.tensor.matmul(
                        den[e][:], lhs, ones[:], start=first, stop=last
                    )
            for e in range(2):
                lj = 2 * p + e
                r = rp.tile([P, 1], F32, tag="r", name="r")
                nc.vector.reciprocal(r[:], den[e][:])
                for n in range(2):
                    osb = op.tile([P, 512], F32, tag="osb", name="osb")
                    nc.vector.tensor_scalar_mul(osb[:], av[e][n][:], r[:])
                    nc.sync.dma_start(
                        O[lj * P : (lj + 1) * P, n * 512 : (n + 1) * 512], osb[:]
                    )

    nc.compile()
    _prog_cache["nc"] = nc
    return nc


def _build_masks(parity: int) -> np.ndarray:
    """[16, 128, 256] bf16 multiplicative masks in S^T layout [k, q].

    Mask iterations (uniform across cores): the last 4 key blocks of each
    pair. Block value: 1 where key_global <= query_global else 0.
    """
    G = G_EVEN if parity == 0 else G_ODD
    out = np.zeros((16, P, 2 * P), dtype=np.float32)
    tri = (np.arange(P)[:, None] <= np.arange(P)[None, :]).astype(np.float32)
    for p in range(4):
        for j in range(4):
            kb = CNT[p] - 4 + j
            for half in range(2):
                g = G[2 * p + half]
                blk = out[p * 4 + j][:, half * P : (half + 1) * P]
                if kb < g:
                    blk[:] = 1.0
                elif kb == g:
                    blk[:] = tri
                # else stays 0
    return out.astype(bf16)


def kernel(x, W_q, W_k, W_v):
    x = np.asarray(x, dtype=np.float32)
    nc = _build_program()

    Wq16 = np.asarray(W_q, dtype=np.float32).astype(bf16)
    Wk16 = np.asarray(W_k, dtype=np.float32).astype(bf16)
    Wv16 = np.asarray(W_v, dtype=np.float32).astype(bf16)
    masks_by_parity = [_build_masks(0), _build_masks(1)]
    qcols = {}
    for e, G in ((0, G_EVEN), (1, G_ODD)):
        qcols[e] = np.concatenate([np.arange(g * P, (g + 1) * P) for g in G])

    in_maps = []
    for c in range(8):
        b, e = c // 2, c % 2
        xTb = x[b].T.astype(bf16)  # [D, S], contiguous via astype copy
        in_maps.append(
            {
                "xT": np.ascontiguousarray(xTb),
                "xTq": np.ascontiguousarray(xTb[:, qcols[e]]),
                "Wq": Wq16,
                "Wk": Wk16,
                "Wv": Wv16,
                "masks": masks_by_parity[e],
            }
        )

    res = run_bass_kernel_spmd(nc, in_maps, core_ids=list(range(8)))

    out = np.empty((x.shape[0], S, D), dtype=np.float32)
    for c in range(8):
        b, e = c // 2, c % 2
        G = G_EVEN if e == 0 else G_ODD
        Oc = res.results[c]["O"]
        for lj, g in enumerate(G):
            out[b, g * P : (g + 1) * P, :] = Oc[lj * P : (lj + 1) * P, :]
    return out


# revision 8
# speedup vs baseline: 262.6293x; 262.6293x over previous
"""Causal single-head attention on 8 Trainium2 NeuronCores.

Problem: x:[4,2048,1024] f32, W_q/W_k/W_v:[1024,1024] f32.
  q,k,v = x@W; scores = q@k^T/sqrt(d) causal-masked; out = softmax(scores)@v.

Sharding: 8 cores = 4 batches x 2 query-shards (SPMD, identical program,
per-core data). Causal load balance: the 16 query blocks (128 rows each) of a
batch are split between its 2 cores as evens/odds of a pairing chosen so both
cores share one uniform per-pair key-block-count profile [4,8,12,16]
(optimal: 40 key-block iterations/core vs 64 dense).

Layout trick: everything is computed via out = lhsT.T @ rhs with x fed
PRE-TRANSPOSED from the host (xT = x[b].T), so the kernel needs no on-chip
transposes at all:
  K^T[d,s] = Wk_blk.T @ xT     (lhsT = Wk block, natural layout)
  Q^T[d,q] = Wq_blk.T @ xTq
  V[s,d]   = xT_blk.T @ Wv
  S^T[k,q] = K^T_blk.T @ Q^T   (scores transposed: softmax key-dim = partition)
  P^T      = exp(S^T/32) * mask      (multiplicative post-exp causal mask, host data)
  denom[q] = P^T_blk.T @ ones  ([q,1] per-partition layout for free)
  out[q,d] = P^T_blk.T @ V     (PSUM-accumulated over key blocks)
  out     *= 1/denom           (per-partition broadcast)

All matmul inputs bf16 (1 cycle/row, FWL), f32 PSUM accumulation.
"""

import numpy as np
import ml_dtypes

import concourse.mybir as mybir
import concourse.tile as tile
from concourse import bacc
from concourse.bass_utils import run_bass_kernel_spmd
from contextlib import ExitStack

P = 128
S = 2048
D = 1024
NIB = D // P  # 8 contraction blocks
NSB = S // P  # 16 key blocks
NQB = 8  # local query blocks per core
CNT = [4, 8, 12, 16]  # key blocks per query-block pair (uniform across cores)
G_EVEN = [0, 2, 4, 6, 9, 11, 13, 15]
G_ODD = [1, 3, 5, 7, 8, 10, 12, 14]
BF = mybir.dt.bfloat16
F32 = mybir.dt.float32
SCALE = 1.0 / 32.0  # 1/sqrt(1024)
bf16 = ml_dtypes.bfloat16

_prog_cache = {}


def _build_program(reps: int = 1):
    if reps in _prog_cache:
        return _prog_cache[reps]
    nc = bacc.Bacc("TRN2", target_bir_lowering=False, debug=False, num_devices=8)
    xT = nc.dram_tensor("xT", [D, S], BF, kind="ExternalInput").ap()
    xTq = nc.dram_tensor("xTq", [D, NQB * P], BF, kind="ExternalInput").ap()
    Wq = nc.dram_tensor("Wq", [D, D], BF, kind="ExternalInput").ap()
    Wk = nc.dram_tensor("Wk", [D, D], BF, kind="ExternalInput").ap()
    Wv = nc.dram_tensor("Wv", [D, D], BF, kind="ExternalInput").ap()
    masks = nc.dram_tensor("masks", [16, P, 2 * P], BF, kind="ExternalInput").ap()
    O = nc.dram_tensor("O", [NQB * P, D], F32, kind="ExternalOutput").ap()

    with tile.TileContext(nc) as tc:
        for _rep in range(reps):
            _emit_body(nc, tc, xT, xTq, Wq, Wk, Wv, masks, O)

    nc.compile()
    _prog_cache[reps] = nc
    return nc


def _emit_body(nc, tc, xT, xTq, Wq, Wk, Wv, masks, O):
    with ExitStack() as ctx:
        # Persistent SBUF residents
        res = ctx.enter_context(tc.tile_pool(name="res", bufs=1))
        kT = [res.tile([P, S], BF, tag=f"kT{d}", name=f"kT{d}") for d in range(NIB)]
        qT = [res.tile([P, NQB * P], BF, tag=f"qT{d}", name=f"qT{d}") for d in range(NIB)]
        v = [res.tile([P, D], BF, tag=f"v{s}", name=f"v{s}") for s in range(NSB)]
        ones = res.tile([P, 1], BF, tag="ones", name="ones")
        nc.vector.memset(ones[:], 1.0)

        # ---------------- Phase A: projections ----------------
        with ExitStack() as actx:
            xp = actx.enter_context(tc.tile_pool(name="xp", bufs=1))
            wp = actx.enter_context(tc.tile_pool(name="wp", bufs=1))
            aps = actx.enter_context(tc.tile_pool(name="aps", bufs=2, space="PSUM"))

            xt = [xp.tile([P, S], BF, tag=f"x{i}", name=f"x{i}") for i in range(NIB)]
            xtq = [xp.tile([P, NQB * P], BF, tag=f"xq{i}", name=f"xq{i}") for i in range(NIB)]
            wk = [wp.tile([P, D], BF, tag=f"wk{i}", name=f"wk{i}") for i in range(NIB)]
            wq = [wp.tile([P, D], BF, tag=f"wq{i}", name=f"wq{i}") for i in range(NIB)]
            wv = [wp.tile([P, D], BF, tag=f"wv{i}", name=f"wv{i}") for i in range(NIB)]
            for i in range(NIB):
                nc.sync.dma_start(xt[i][:], xT[i * P : (i + 1) * P, :])
                nc.sync.dma_start(wk[i][:], Wk[i * P : (i + 1) * P, :])
                nc.sync.dma_start(xtq[i][:], xTq[i * P : (i + 1) * P, :])
                nc.sync.dma_start(wq[i][:], Wq[i * P : (i + 1) * P, :])
                nc.sync.dma_start(wv[i][:], Wv[i * P : (i + 1) * P, :])

            def proj(dst, dst_col, lhsT, rhs, n512):
                """dst[:, dst_col*512+...] = sum_i lhsT[i].T @ rhs[i], n512 cols"""
                ps = aps.tile([P, 512], F32, tag="aps", name="aps")
                for i in range(NIB):
                    nc.tensor.matmul(
                        ps[:], lhsT[i], rhs[i], start=(i == 0), stop=(i == NIB - 1)
                    )
                nc.vector.tensor_copy(
                    dst[:, dst_col * 512 : dst_col * 512 + 512], ps[:]
                )

            # K^T then Q^T (phase B pair 0 needs them first), then V
            for d in range(NIB):
                for n in range(S // 512):
                    proj(
                        kT[d], n,
                        [wk[i][:, d * P : (d + 1) * P] for i in range(NIB)],
                        [xt[i][:, n * 512 : (n + 1) * 512] for i in range(NIB)],
                        1,
                    )
            for d in range(NIB):
                for n in range(NQB * P // 512):
                    proj(
                        qT[d], n,
                        [wq[i][:, d * P : (d + 1) * P] for i in range(NIB)],
                        [xtq[i][:, n * 512 : (n + 1) * 512] for i in range(NIB)],
                        1,
                    )
            for s in range(NSB):
                for n in range(D // 512):
                    proj(
                        v[s], n,
                        [xt[i][:, s * P : (s + 1) * P] for i in range(NIB)],
                        [wv[i][:, n * 512 : (n + 1) * 512] for i in range(NIB)],
                        1,
                    )

        # ---------------- Phase B: attention ----------------
        mp = ctx.enter_context(tc.tile_pool(name="mp", bufs=1))
        m_tiles = [mp.tile([P, 2 * P], BF, tag=f"m{i}", name=f"m{i}") for i in range(16)]
        for i in range(16):
            nc.sync.dma_start(m_tiles[i][:], masks[i, :, :])

        spool = ctx.enter_context(tc.tile_pool(name="spool", bufs=2, space="PSUM"))
        avpool = ctx.enter_context(tc.tile_pool(name="avpool", bufs=1, space="PSUM"))
        dpool = ctx.enter_context(tc.tile_pool(name="dpool", bufs=1, space="PSUM"))
        pp = ctx.enter_context(tc.tile_pool(name="pp", bufs=3))
        op = ctx.enter_context(tc.tile_pool(name="op", bufs=2))
        rp = ctx.enter_context(tc.tile_pool(name="rp", bufs=2))

        for p in range(4):
            av = [
                [avpool.tile([P, 512], F32, tag=f"av{e}{n}", name=f"av{e}{n}") for n in range(2)]
                for e in range(2)
            ]
            # one PSUM tile (bank) per query-half: matmul start=True clears
            # has_written for the WHOLE bank, so two interleaved accumulation
            # groups cannot share a bank
            den = [
                dpool.tile([P, 1], F32, tag=f"den{e}", name=f"den{e}")
                for e in range(2)
            ]
            for kb in range(CNT[p]):
                ps_s = spool.tile([P, 2 * P], F32, tag="ps_s", name="ps_s")
                for d in range(NIB):
                    nc.tensor.matmul(
                        ps_s[:],
                        kT[d][:, kb * P : (kb + 1) * P],
                        qT[d][:, p * 2 * P : (p + 1) * 2 * P],
                        start=(d == 0),
                        stop=(d == NIB - 1),
                    )
                pT = pp.tile([P, 2 * P], BF, tag="pT", name="pT")
                nc.scalar.activation(
                    pT[:], ps_s[:], mybir.ActivationFunctionType.Exp, scale=SCALE
                )
                if kb >= CNT[p] - 4:
                    mi = p * 4 + kb - (CNT[p] - 4)
                    pTm = pp.tile([P, 2 * P], BF, tag="pTm", name="pTm")
                    nc.vector.tensor_mul(pTm[:], pT[:], m_tiles[mi][:])
                    pT = pTm
                first, last = (kb == 0), (kb == CNT[p] - 1)
                for e in range(2):
                    lhs = pT[:, e * P : (e + 1) * P]
                    for n in range(2):
                        nc.tensor.matmul(
                            av[e][n][:], lhs, v[kb][:, n * 512 : (n + 1) * 512],
                            start=first, stop=last,
                        )
                    nc.tensor.matmul(
                        den[e][:], lhs, ones[:], start=first, stop=last
                    )
            for e in range(2):
                lj = 2 * p + e
                r = rp.tile([P, 1], F32, tag="r", name="r")
                nc.vector.reciprocal(r[:], den[e][:])
                for n in range(2):
                    osb = op.tile([P, 512], F32, tag="osb", name="osb")
                    nc.vector.tensor_scalar_mul(osb[:], av[e][n][:], r[:])
                    nc.sync.dma_start(
                        O[lj * P : (lj + 1) * P, n * 512 : (n + 1) * 512], osb[:]
                    )


def _build_masks(parity: int) -> np.ndarray:
    """[16, 128, 256] bf16 multiplicative masks in S^T layout [k, q].

    Mask iterations (uniform across cores): the last 4 key blocks of each
    pair. Block value: 1 where key_global <= query_global else 0.
    """
    G = G_EVEN if parity == 0 else G_ODD
    out = np.zeros((16, P, 2 * P), dtype=np.float32)
    tri = (np.arange(P)[:, None] <= np.arange(P)[None, :]).astype(np.float32)
    for p in range(4):
        for j in range(4):
            kb = CNT[p] - 4 + j
            for half in range(2):
                g = G[2 * p + half]
                blk = out[p * 4 + j][:, half * P : (half + 1) * P]
                if kb < g:
                    blk[:] = 1.0
                elif kb == g:
                    blk[:] = tri
                # else stays 0
    return out.astype(bf16)


def kernel(x, W_q, W_k, W_v):
    x = np.asarray(x, dtype=np.float32)
    nc = _build_program()

    Wq16 = np.asarray(W_q, dtype=np.float32).astype(bf16)
    Wk16 = np.asarray(W_k, dtype=np.float32).astype(bf16)
    Wv16 = np.asarray(W_v, dtype=np.float32).astype(bf16)
    masks_by_parity = [_build_masks(0), _build_masks(1)]
    qcols = {}
    for e, G in ((0, G_EVEN), (1, G_ODD)):
        qcols[e] = np.concatenate([np.arange(g * P, (g + 1) * P) for g in G])

    in_maps = []
    for c in range(8):
        b, e = c // 2, c % 2
        xTb = x[b].T.astype(bf16)  # [D, S], contiguous via astype copy
        in_maps.append(
            {
                "xT": np.ascontiguousarray(xTb),
                "xTq": np.ascontiguousarray(xTb[:, qcols[e]]),
                "Wq": Wq16,
                "Wk": Wk16,
                "Wv": Wv16,
                "masks": masks_by_parity[e],
            }
        )

    res = run_bass_kernel_spmd(nc, in_maps, core_ids=list(range(8)))

    out = np.empty((x.shape[0], S, D), dtype=np.float32)
    for c in range(8):
        b, e = c // 2, c % 2
        G = G_EVEN if e == 0 else G_ODD
        Oc = res.results[c]["O"]
        for lj, g in enumerate(G):
            out[b, g * P : (g + 1) * P, :] = Oc[lj * P : (lj + 1) * P, :]
    return out


# revision 11
# speedup vs baseline: 410.4078x; 1.5627x over previous
"""Causal single-head attention on 8 Trainium2 NeuronCores.

Problem: x:[4,2048,1024] f32, W_q/W_k/W_v:[1024,1024] f32.
  q,k,v = x@W; scores = q@k^T/sqrt(d) causal-masked; out = softmax(scores)@v.

Sharding: 8 cores = 4 batches x 2 query-shards (SPMD, identical program,
per-core data). Causal load balance: the 16 query blocks (128 rows each) of a
batch are split between its 2 cores as evens/odds of a pairing chosen so both
cores share one uniform per-pair key-block-count profile [4,8,12,16]
(optimal: 40 key-block iterations/core vs 64 dense).

Layout trick: everything is computed via out = lhsT.T @ rhs with x fed
PRE-TRANSPOSED from the host (xT = x[b].T), so the kernel needs no on-chip
transposes at all:
  K^T[d,s] = Wk_blk.T @ xT     (lhsT = Wk block, natural layout)
  Q^T[d,q] = Wq_blk.T @ xTq
  V[s,d]   = xT_blk.T @ Wv
  S^T[k,q] = K^T_blk.T @ Q^T   (scores transposed: softmax key-dim = partition)
  P^T      = exp(S^T/32) * mask      (multiplicative post-exp causal mask, host data)
  denom[q] = P^T_blk.T @ ones  ([q,1] per-partition layout for free)
  out[q,d] = P^T_blk.T @ V     (PSUM-accumulated over key blocks)
  out     *= 1/denom           (per-partition broadcast)

All matmul inputs bf16 (1 cycle/row, FWL), f32 PSUM accumulation.
"""

import numpy as np
import ml_dtypes

import concourse.mybir as mybir
import concourse.tile as tile
from concourse import bacc
from concourse.bass_utils import run_bass_kernel_spmd
from contextlib import ExitStack

P = 128
S = 2048
D = 1024
NIB = D // P  # 8 contraction blocks
NSB = S // P  # 16 key blocks
NQB = 8  # local query blocks per core
CNT = [4, 8, 12, 16]  # key blocks per query-block pair (uniform across cores)
G_EVEN = [0, 2, 4, 6, 9, 11, 13, 15]
G_ODD = [1, 3, 5, 7, 8, 10, 12, 14]
BF = mybir.dt.bfloat16
F32 = mybir.dt.float32
SCALE = 1.0 / 32.0  # 1/sqrt(1024)
bf16 = ml_dtypes.bfloat16

_prog_cache = {}


def _build_program(reps: int = 1, parts: str = "all"):
    key = (reps, parts)
    if key in _prog_cache:
        return _prog_cache[key]
    nc = bacc.Bacc("TRN2", target_bir_lowering=False, debug=False, num_devices=8)
    xT = nc.dram_tensor("xT", [D, S], BF, kind="ExternalInput").ap()
    xTq = nc.dram_tensor("xTq", [D, NQB * P], BF, kind="ExternalInput").ap()
    Wq = nc.dram_tensor("Wq", [D, D], BF, kind="ExternalInput").ap()
    Wk = nc.dram_tensor("Wk", [D, D], BF, kind="ExternalInput").ap()
    Wv = nc.dram_tensor("Wv", [D, D], BF, kind="ExternalInput").ap()
    masks = nc.dram_tensor("masks", [16, P, 2 * P], BF, kind="ExternalInput").ap()
    O = nc.dram_tensor("O", [NQB * P, D], F32, kind="ExternalOutput").ap()

    with tile.TileContext(nc) as tc:
        for _rep in range(reps):
            _emit_body(nc, tc, xT, xTq, Wq, Wk, Wv, masks, O, parts)

    nc.compile()
    _prog_cache[key] = nc
    return nc


def _emit_body(nc, tc, xT, xTq, Wq, Wk, Wv, masks, O, parts="all"):
    with ExitStack() as ctx:
        # Persistent SBUF residents
        res = ctx.enter_context(tc.tile_pool(name="res", bufs=1))
        kT = [res.tile([P, S], BF, tag=f"kT{d}", name=f"kT{d}") for d in range(NIB)]
        qT = [res.tile([P, NQB * P], BF, tag=f"qT{d}", name=f"qT{d}") for d in range(NIB)]
        v = [res.tile([P, D], BF, tag=f"v{s}", name=f"v{s}") for s in range(NSB)]
        ones = res.tile([P, 1], BF, tag="ones", name="ones")
        nc.vector.memset(ones[:], 1.0)

        # ---------------- Phase A: projections ----------------
        with ExitStack() as actx:
            xp = actx.enter_context(tc.tile_pool(name="xp", bufs=1))
            wp = actx.enter_context(tc.tile_pool(name="wp", bufs=1))
            aps = actx.enter_context(tc.tile_pool(name="aps", bufs=2, space="PSUM"))

            xt = [xp.tile([P, S], BF, tag=f"x{i}", name=f"x{i}") for i in range(NIB)]
            xtq = [xp.tile([P, NQB * P], BF, tag=f"xq{i}", name=f"xq{i}") for i in range(NIB)]
            wk = [wp.tile([P, D], BF, tag=f"wk{i}", name=f"wk{i}") for i in range(NIB)]
            wq = [wp.tile([P, D], BF, tag=f"wq{i}", name=f"wq{i}") for i in range(NIB)]
            wv = [wp.tile([P, D], BF, tag=f"wv{i}", name=f"wv{i}") for i in range(NIB)]
            for i in range(NIB):
                nc.sync.dma_start(xt[i][:], xT[i * P : (i + 1) * P, :])
                nc.sync.dma_start(wk[i][:], Wk[i * P : (i + 1) * P, :])
                nc.sync.dma_start(xtq[i][:], xTq[i * P : (i + 1) * P, :])
                nc.sync.dma_start(wq[i][:], Wq[i * P : (i + 1) * P, :])
                nc.sync.dma_start(wv[i][:], Wv[i * P : (i + 1) * P, :])

            def proj(dst, dst_col, lhsT, rhs, n512):
                """dst[:, dst_col*512+...] = sum_i lhsT[i].T @ rhs[i], n512 cols"""
                ps = aps.tile([P, 512], F32, tag="aps", name="aps")
                for i in range(NIB):
                    nc.tensor.matmul(
                        ps[:], lhsT[i], rhs[i], start=(i == 0), stop=(i == NIB - 1)
                    )
                nc.vector.tensor_copy(
                    dst[:, dst_col * 512 : dst_col * 512 + 512], ps[:]
                )

            # K^T then Q^T (phase B pair 0 needs them first), then V
            for d in range(NIB):
                for n in range(S // 512):
                    proj(
                        kT[d], n,
                        [wk[i][:, d * P : (d + 1) * P] for i in range(NIB)],
                        [xt[i][:, n * 512 : (n + 1) * 512] for i in range(NIB)],
                        1,
                    )
            for d in range(NIB):
                for n in range(NQB * P // 512):
                    proj(
                        qT[d], n,
                        [wq[i][:, d * P : (d + 1) * P] for i in range(NIB)],
                        [xtq[i][:, n * 512 : (n + 1) * 512] for i in range(NIB)],
                        1,
                    )
            for s in range(NSB):
                for n in range(D // 512):
                    proj(
                        v[s], n,
                        [xt[i][:, s * P : (s + 1) * P] for i in range(NIB)],
                        [wv[i][:, n * 512 : (n + 1) * 512] for i in range(NIB)],
                        1,
                    )

        if parts == "A":
            # timing variant: keep phase A alive via strip copies + one DMA out
            with tc.tile_pool(name="ka", bufs=1) as ka:
                coll = ka.tile([P, 4 * (NIB * 6 + NSB * 2)], F32, name="coll")
                col = 0
                for d in range(NIB):
                    for n in range(4):
                        nc.vector.tensor_copy(
                            coll[:, col : col + 4], kT[d][:, n * 512 : n * 512 + 4]
                        )
                        col += 4
                    for n in range(2):
                        nc.vector.tensor_copy(
                            coll[:, col : col + 4], qT[d][:, n * 512 : n * 512 + 4]
                        )
                        col += 4
                for s in range(NSB):
                    for n in range(2):
                        nc.vector.tensor_copy(
                            coll[:, col : col + 4], v[s][:, n * 512 : n * 512 + 4]
                        )
                        col += 4
                nc.sync.dma_start(O[0:P, 0:col], coll[:, 0:col])
            return

        # ---------------- Phase B: attention ----------------
        mp = ctx.enter_context(tc.tile_pool(name="mp", bufs=1))
        m_tiles = [mp.tile([P, 2 * P], BF, tag=f"m{i}", name=f"m{i}") for i in range(16)]
        for i in range(16):
            nc.sync.dma_start(m_tiles[i][:], masks[i, :, :])

        spool = ctx.enter_context(tc.tile_pool(name="spool", bufs=2, space="PSUM"))
        avpool = ctx.enter_context(tc.tile_pool(name="avpool", bufs=1, space="PSUM"))
        dpool = ctx.enter_context(tc.tile_pool(name="dpool", bufs=1, space="PSUM"))
        pp = ctx.enter_context(tc.tile_pool(name="pp", bufs=3))
        op = ctx.enter_context(tc.tile_pool(name="op", bufs=2))
        rp = ctx.enter_context(tc.tile_pool(name="rp", bufs=2))

        for p in range(4):
            av = [
                [avpool.tile([P, 512], F32, tag=f"av{e}{n}", name=f"av{e}{n}") for n in range(2)]
                for e in range(2)
            ]
            # one PSUM tile (bank) per query-half: matmul start=True clears
            # has_written for the WHOLE bank, so two interleaved accumulation
            # groups cannot share a bank
            den = [
                dpool.tile([P, 1], F32, tag=f"den{e}", name=f"den{e}")
                for e in range(2)
            ]
            for kb in range(CNT[p]):
                ps_s = spool.tile([P, 2 * P], F32, tag="ps_s", name="ps_s")
                for d in range(NIB):
                    nc.tensor.matmul(
                        ps_s[:],
                        kT[d][:, kb * P : (kb + 1) * P],
                        qT[d][:, p * 2 * P : (p + 1) * 2 * P],
                        start=(d == 0),
                        stop=(d == NIB - 1),
                    )
                pT = pp.tile([P, 2 * P], BF, tag="pT", name="pT")
                nc.scalar.activation(
                    pT[:], ps_s[:], mybir.ActivationFunctionType.Exp, scale=SCALE
                )
                if kb >= CNT[p] - 4:
                    mi = p * 4 + kb - (CNT[p] - 4)
                    pTm = pp.tile([P, 2 * P], BF, tag="pTm", name="pTm")
                    nc.vector.tensor_mul(pTm[:], pT[:], m_tiles[mi][:])
                    pT = pTm
                first, last = (kb == 0), (kb == CNT[p] - 1)
                for e in range(2):
                    lhs = pT[:, e * P : (e + 1) * P]
                    for n in range(2):
                        nc.tensor.matmul(
                            av[e][n][:], lhs, v[kb][:, n * 512 : (n + 1) * 512],
                            start=first, stop=last,
                        )
                    nc.tensor.matmul(
                        den[e][:], lhs, ones[:], start=first, stop=last
                    )
            for e in range(2):
                lj = 2 * p + e
                r = rp.tile([P, 1], F32, tag="r", name="r")
                nc.vector.reciprocal(r[:], den[e][:])
                for n in range(2):
                    osb = op.tile([P, 512], F32, tag="osb", name="osb")
                    nc.vector.tensor_scalar_mul(osb[:], av[e][n][:], r[:])
                    nc.sync.dma_start(
                        O[lj * P : (lj + 1) * P, n * 512 : (n + 1) * 512], osb[:]
                    )


def _build_masks(parity: int) -> np.ndarray:
    """[16, 128, 256] bf16 multiplicative masks in S^T layout [k, q].

    Mask iterations (uniform across cores): the last 4 key blocks of each
    pair. Block value: 1 where key_global <= query_global else 0.
    """
    G = G_EVEN if parity == 0 else G_ODD
    out = np.zeros((16, P, 2 * P), dtype=np.float32)
    tri = (np.arange(P)[:, None] <= np.arange(P)[None, :]).astype(np.float32)
    for p in range(4):
        for j in range(4):
            kb = CNT[p] - 4 + j
            for half in range(2):
                g = G[2 * p + half]
                blk = out[p * 4 + j][:, half * P : (half + 1) * P]
                if kb < g:
                    blk[:] = 1.0
                elif kb == g:
                    blk[:] = tri
                # else stays 0
    return out.astype(bf16)


def kernel(x, W_q, W_k, W_v):
    x = np.asarray(x, dtype=np.float32)
    nc = _build_program()

    Wq16 = np.asarray(W_q, dtype=np.float32).astype(bf16)
    Wk16 = np.asarray(W_k, dtype=np.float32).astype(bf16)
    Wv16 = np.asarray(W_v, dtype=np.float32).astype(bf16)
    masks_by_parity = [_build_masks(0), _build_masks(1)]
    qcols = {}
    for e, G in ((0, G_EVEN), (1, G_ODD)):
        qcols[e] = np.concatenate([np.arange(g * P, (g + 1) * P) for g in G])

    in_maps = []
    for c in range(8):
        b, e = c // 2, c % 2
        xTb = x[b].T.astype(bf16)  # [D, S], contiguous via astype copy
        in_maps.append(
            {
                "xT": np.ascontiguousarray(xTb),
                "xTq": np.ascontiguousarray(xTb[:, qcols[e]]),
                "Wq": Wq16,
                "Wk": Wk16,
                "Wv": Wv16,
                "masks": masks_by_parity[e],
            }
        )

    res = run_bass_kernel_spmd(nc, in_maps, core_ids=list(range(8)))

    out = np.empty((x.shape[0], S, D), dtype=np.float32)
    for c in range(8):
        b, e = c // 2, c % 2
        G = G_EVEN if e == 0 else G_ODD
        Oc = res.results[c]["O"]
        for lj, g in enumerate(G):
            out[b, g * P : (g + 1) * P, :] = Oc[lj * P : (lj + 1) * P, :]
    return out
